# revision 7
# baseline (speedup 1.0000x reference)
"""Trainium2 Bass kernel for DequantingLinear (GGML Q8_0 dequant + linear).

Computes out[4096, 12288] = x[4096, 3072] @ dequant(w_q, w_scales).T + bias
where w_q is int32 (int8-valued) with per-32-element-block fp32 scales.

Sharding: tensor-parallel over output features across 8 NeuronCores. Each
core gets the full x and a 1536-row shard of w_q / w_scales / bias,
computes its [4096, 1536] output slice; the host concatenates on axis 1.

Per-core kernel (Tile framework):
  * w path: contiguous [128out, 3072] int32 loads -> DVE dequant
    (block scales broadcast over QK=32, int32 x fp32 -> bf16, exact for
    |q|<=127) -> PE-array transposes (8 k-tiles per [128,8,128] bf16 PSUM
    tile via an identity ifmap) -> ACT copy into the resident
    [in, k, out] SBUF weight tensor. No DRAM bounce and no DMA-xbar
    transposes on the w path (DMA-xbar descriptors measured ~2x the
    cost-model estimate on HW; the PE route is cheap for w's 4.7M elems).
  * x path: fp32 -> bf16 DRAM->DRAM SWDGE casts into a 2-slot ring of
    1024-token blocks (halving transpose-DMA count vs 512 blocks), then
    per-(k-tile, block) DMA-xbar transpose loads into [in, k, tok] SBUF
    tiles. (x is 2.7x larger than w; routing it through the PE was
    measured slower than the xbar because PE transposes interleaved with
    the GEMM cost ~5x their isolated rate.)
  * GEMM: psum[128tok, 512out] fp32 tiles accumulate 24 bf16 k-tile
    matmuls; bias added during the PSUM->SBUF drain on the vector engine;
    the three 512-wide column chunks of each tok-128 row are merged into
    one [128, 1536] staging tile and written with a single out-DMA.
  * Schedule: staged head (w og0-3 + x b0 -> gemm b0/n0 -> og4-7 + x b1
    -> gemm b1/n0 -> og8-11) so the PE ramps while w-prep streams.
  All HWDGE DMAs are issued on nc.sync (ACT-issued DMAs were observed to
  produce corrupted results on hardware in this configuration).
"""

import sys

for _p in ("/opt/trn_rl_repo", "/root/problem"):
    if _p not in sys.path:
        sys.path.append(_p)

import numpy as np
from contextlib import ExitStack

import concourse.bass as bass
import concourse.mybir as mybir
from concourse import tile
from concourse.tile_rust import add_dep_helper

FP32 = mybir.dt.float32
BF16 = mybir.dt.bfloat16
INT32 = mybir.dt.int32

N_CORES = 8
TOK, IN, OUT = 4096, 3072, 12288
QK = 32
OUT_SH = OUT // N_CORES
TOK_BLK = 1024
NCOL = 512
RING_SLOTS = 2

P = 128
KT = IN // P
NBLK = TOK // TOK_BLK
MT = TOK_BLK // P
NT = OUT_SH // NCOL
NB = IN // QK
OT = OUT_SH // P
KG = 8
NKG = KT // KG
CHUNK = KG * P


def _build(nc: bass.Bass, repeats: int = 1):
    x = nc.dram_tensor("x", [TOK, IN], FP32, kind="ExternalInput")
    w_q = nc.dram_tensor("w_q", [OUT_SH, IN], INT32, kind="ExternalInput")
    w_scales = nc.dram_tensor("w_scales", [OUT_SH, NB], FP32, kind="ExternalInput")
    bias = nc.dram_tensor("bias", [OUT_SH], FP32, kind="ExternalInput")
    out = nc.dram_tensor("out", [TOK, OUT_SH], FP32, kind="ExternalOutput")

    x_slots = [
        nc.dram_tensor(f"x_bf16_{s}", [TOK_BLK, IN], BF16) for s in range(RING_SLOTS)
    ]

    with tile.TileContext(nc) as tc, ExitStack() as ctx:
        const_pool = ctx.enter_context(tc.tile_pool(name="const", bufs=1))
        wq_pool = ctx.enter_context(tc.tile_pool(name="wq", bufs=3))
        wd_pool = ctx.enter_context(tc.tile_pool(name="wd", bufs=2))
        wt_pool = ctx.enter_context(tc.tile_pool(name="wt", bufs=1))
        xt_pool = ctx.enter_context(tc.tile_pool(name="xt", bufs=2))
        obs_pool = ctx.enter_context(tc.tile_pool(name="obs", bufs=2))
        obm_pool = ctx.enter_context(tc.tile_pool(name="obm", bufs=2))
        psum_pool = ctx.enter_context(tc.tile_pool(name="psum", bufs=6, space="PSUM"))
        tr_pool = ctx.enter_context(tc.tile_pool(name="tr", bufs=2, space="PSUM"))

        ident = const_pool.tile([P, P], BF16, tag="ident")
        nc.gpsimd.memset(ident[:], 1.0)
        nc.gpsimd.affine_select(
            ident[:], ident[:], [[-1, P]], mybir.AluOpType.is_equal, 0.0,
            channel_multiplier=1,
        )

        for _rep in range(repeats):
            _rep_body(
                nc, tc, const_pool, wq_pool, wd_pool, wt_pool, xt_pool,
                obs_pool, obm_pool, psum_pool, tr_pool, ident,
                x, w_q, w_scales, bias, out, x_slots,
            )
    return nc


def _rep_body(
    nc, tc, const_pool, wq_pool, wd_pool, wt_pool, xt_pool, obs_pool,
    obm_pool, psum_pool, tr_pool, ident, x, w_q, w_scales, bias, out, x_slots,
):
    wt = wt_pool.tile([P, KT, OUT_SH], BF16, tag="wt")
    sc_pool = const_pool

    def w_og(og):
        rows = slice(og * P, (og + 1) * P)
        sct = sc_pool.tile([P, NB], FP32, tag="sc", name=f"sc_{og}", bufs=2)
        nc.sync.dma_start(sct[:], w_scales.ap()[rows, :])
        for kg in range(NKG):
            cs = slice(kg * CHUNK, (kg + 1) * CHUNK)
            wq_i = wq_pool.tile([P, CHUNK], INT32, tag="wq", name=f"wq_{og}_{kg}")
            nc.sync.dma_start(wq_i[:], w_q.ap()[rows, cs])
            wd = wd_pool.tile([P, CHUNK], BF16, tag="wd")
            nc.vector.tensor_mul(
                wd[:].rearrange("p (b q) -> p b q", q=QK),
                wq_i[:].rearrange("p (b q) -> p b q", q=QK),
                sct[:, kg * (CHUNK // QK) : (kg + 1) * (CHUNK // QK)]
                .unsqueeze(2)
                .to_broadcast([P, CHUNK // QK, QK]),
            )
            tr = tr_pool.tile([P, KG, P], BF16, tag="tr")
            for j in range(KG):
                nc.tensor.transpose(
                    tr[:, j, :], wd[:, j * P : (j + 1) * P], ident[:]
                )
            nc.scalar.copy(
                wt[:, kg * KG : (kg + 1) * KG, og * P : (og + 1) * P], tr[:]
            )

    def cast_block(b):
        s = b % RING_SLOTS
        srows = slice(b * TOK_BLK, (b + 1) * TOK_BLK)
        return nc.gpsimd.dma_start(x_slots[s].ap()[:, :], x.ap()[srows, :])

    def load_xt(b):
        s = b % RING_SLOTS
        xt = xt_pool.tile([P, KT, TOK_BLK], BF16, tag="xt", name=f"xt_{b}")
        for k in range(KT):
            nc.sync.dma_start(
                xt[:, k, :],
                x_slots[s].ap()[:, k * P : (k + 1) * P],
                transpose=True,
            )
        return xt

    bias_rep = None

    obm_tiles = {}

    def gemm_group(xt, b, m, n, ns):
        tok0 = b * TOK_BLK + m * P
        ps = psum_pool.tile([P, NCOL], FP32, tag="ps")
        for k in range(KT):
            nc.tensor.matmul(
                ps[:],
                xt[:, k, m * P : (m + 1) * P],
                wt[:, k, n * NCOL : (n + 1) * NCOL],
                start=(k == 0),
                stop=(k == KT - 1),
            )
        if n == 0:
            ob = obs_pool.tile([P, NCOL], FP32, tag="obs", name=f"obs_{b}_{m}")
            nc.vector.tensor_add(ob[:], ps[:], bias_rep[:, 0:NCOL])
            nc.sync.dma_start(out.ap()[tok0 : tok0 + P, 0:NCOL], ob[:])
            return
        if (b, m) not in obm_tiles:
            obm_tiles[(b, m)] = obm_pool.tile(
                [P, 2 * NCOL], FP32, tag="obm", name=f"obm_{b}_{m}"
            )
        ob = obm_tiles[(b, m)]
        c0 = (n - 1) * NCOL
        nc.vector.tensor_add(
            ob[:, c0 : c0 + NCOL],
            ps[:],
            bias_rep[:, n * NCOL : (n + 1) * NCOL],
        )
        if n == 2:
            del obm_tiles[(b, m)]
            nc.sync.dma_start(
                out.ap()[tok0 : tok0 + P, NCOL : 3 * NCOL], ob[:]
            )

    # --- stage A: og0-3 + x b0 ---
    cast_block(0)
    cast_block(1)
    for i in range(4):
        w_og(i)
    xt_cache = {0: load_xt(0)}

    bias_rep = const_pool.tile([P, OUT_SH], BF16, tag="bias_rep")
    nc.gpsimd.dma_start(bias_rep[:], bias.ap().unsqueeze(0).to_broadcast([P, OUT_SH]))

    for m in range(MT):
        gemm_group(xt_cache[0], 0, m, 0, [0, 1, 2])

    # --- stage B: og4-7 + x b1 ---
    for i in range(4):
        w_og(4 + i)
    xt_cache[1] = load_xt(1)

    for m in range(MT):
        gemm_group(xt_cache[1], 1, m, 0, [0, 1, 2])

    # --- stage C: og8-11 ---
    last_w = None
    for i in range(4):
        w_og(8 + i)

    # --- main loop ---
    ncast = 2
    for b in range(NBLK):
        if b in xt_cache:
            xt = xt_cache.pop(b)
        else:
            xt = load_xt(b)
        if ncast < NBLK:
            cast_block(ncast)
            ncast += 1
        ns = [1, 2] if b < 2 else [0, 1, 2]
        for m in range(MT):
            for n in ns:
                gemm_group(xt, b, m, n, ns)


_COMPILED_NC = None


def _get_nc():
    global _COMPILED_NC
    if _COMPILED_NC is None:
        import concourse.bacc as bacc

        nc = bacc.Bacc("TRN2", target_bir_lowering=False, debug=False)
        _build(nc)
        nc.compile()
        _COMPILED_NC = nc
    return _COMPILED_NC


def kernel(x, w_q, w_scales, bias):
    from concourse.bass_utils import run_bass_kernel_spmd

    assert x.shape == (TOK, IN) and w_q.shape == (OUT, IN)
    nc = _get_nc()
    x = np.ascontiguousarray(np.asarray(x, dtype=np.float32))
    w_q = np.asarray(w_q, dtype=np.int32)
    w_scales = np.asarray(w_scales, dtype=np.float32)
    bias = np.asarray(bias, dtype=np.float32)
    in_maps = []
    for c in range(N_CORES):
        r = slice(c * OUT_SH, (c + 1) * OUT_SH)
        in_maps.append(
            {
                "x": x,
                "w_q": np.ascontiguousarray(w_q[r]),
                "w_scales": np.ascontiguousarray(w_scales[r]),
                "bias": np.ascontiguousarray(bias[r]),
            }
        )
    res = run_bass_kernel_spmd(nc, in_maps, list(range(N_CORES)))
    return np.concatenate([res.results[c]["out"] for c in range(N_CORES)], axis=1)


# revision 8
# speedup vs baseline: 1.0685x; 1.0685x over previous
"""Trainium2 Bass kernel for DequantingLinear (GGML Q8_0 dequant + linear).

Computes out[4096, 12288] = x[4096, 3072] @ dequant(w_q, w_scales).T + bias
where w_q is int32 (int8-valued) with per-32-element-block fp32 scales.

Sharding: tensor-parallel over output features across 8 NeuronCores. Each
core gets the full x and a 1536-row shard of w_q / w_scales / bias,
computes its [4096, 1536] output slice; the host concatenates on axis 1.

Per-core kernel (Tile framework):
  * w path: contiguous [128out, 3072] int32 loads -> DVE dequant
    (block scales broadcast over QK=32, int32 x fp32 -> bf16, exact for
    |q|<=127) -> PE-array transposes (8 k-tiles per [128,8,128] bf16 PSUM
    tile via an identity ifmap) -> ACT copy into the resident
    [in, k, out] SBUF weight tensor. No DRAM bounce and no DMA-xbar
    transposes on the w path (DMA-xbar descriptors measured ~2x the
    cost-model estimate on HW; the PE route is cheap for w's 4.7M elems).
  * x path: fp32 -> bf16 DRAM->DRAM SWDGE casts into a 4-slot ring, then
    per-(k-tile, block) DMA-xbar transpose loads into [in, k, tok] SBUF
    tiles. (x is 2.7x larger than w; routing it through the PE was
    measured slower than the xbar because PE transposes interleaved with
    the GEMM cost ~5x their isolated rate.)
  * GEMM: psum[128tok, 512out] fp32 tiles accumulate 24 bf16 k-tile
    matmuls; bias added during the PSUM->SBUF drain on the vector engine;
    the three 512-wide column chunks of each tok-128 row are merged into
    one [128, 1536] staging tile and written with a single out-DMA.
  * Schedule: staged head (w og0-3 + x b0 -> gemm b0/n0 -> og4-7 + x b1
    -> gemm b1/n0 -> og8-11) so the PE ramps while w-prep streams.
  All HWDGE DMAs are issued on nc.sync (ACT-issued DMAs were observed to
  produce corrupted results on hardware in this configuration).
"""

import sys

for _p in ("/opt/trn_rl_repo", "/root/problem"):
    if _p not in sys.path:
        sys.path.append(_p)

import numpy as np
from contextlib import ExitStack

import concourse.bass as bass
import concourse.mybir as mybir
from concourse import tile
from concourse.tile_rust import add_dep_helper

FP32 = mybir.dt.float32
BF16 = mybir.dt.bfloat16
INT32 = mybir.dt.int32

N_CORES = 8
TOK, IN, OUT = 4096, 3072, 12288
QK = 32
OUT_SH = OUT // N_CORES
TOK_BLK = 512
NCOL = 512
RING_SLOTS = 4

P = 128
KT = IN // P
NBLK = TOK // TOK_BLK
MT = TOK_BLK // P
NT = OUT_SH // NCOL
NB = IN // QK
OT = OUT_SH // P
KG = 8
NKG = KT // KG
CHUNK = KG * P


def _build(nc: bass.Bass, repeats: int = 1):
    x = nc.dram_tensor("x", [TOK, IN], FP32, kind="ExternalInput")
    w_q = nc.dram_tensor("w_q", [OUT_SH, IN], INT32, kind="ExternalInput")
    w_scales = nc.dram_tensor("w_scales", [OUT_SH, NB], FP32, kind="ExternalInput")
    bias = nc.dram_tensor("bias", [OUT_SH], FP32, kind="ExternalInput")
    out = nc.dram_tensor("out", [TOK, OUT_SH], FP32, kind="ExternalOutput")

    x_slots = [
        nc.dram_tensor(f"x_bf16_{s}", [TOK_BLK, IN], BF16) for s in range(RING_SLOTS)
    ]

    with tile.TileContext(nc) as tc, ExitStack() as ctx:
        const_pool = ctx.enter_context(tc.tile_pool(name="const", bufs=1))
        wq_pool = ctx.enter_context(tc.tile_pool(name="wq", bufs=2))
        wd_pool = ctx.enter_context(tc.tile_pool(name="wd", bufs=2))
        wt_pool = ctx.enter_context(tc.tile_pool(name="wt", bufs=1))
        xt_pool = ctx.enter_context(tc.tile_pool(name="xt", bufs=2))
        out_pool = ctx.enter_context(tc.tile_pool(name="out", bufs=5))
        psum_pool = ctx.enter_context(tc.tile_pool(name="psum", bufs=6, space="PSUM"))
        tr_pool = ctx.enter_context(tc.tile_pool(name="tr", bufs=2, space="PSUM"))

        ident = const_pool.tile([P, P], BF16, tag="ident")
        nc.gpsimd.memset(ident[:], 1.0)
        nc.gpsimd.affine_select(
            ident[:], ident[:], [[-1, P]], mybir.AluOpType.is_equal, 0.0,
            channel_multiplier=1,
        )

        for _rep in range(repeats):
            _rep_body(
                nc, tc, const_pool, wq_pool, wd_pool, wt_pool, xt_pool,
                out_pool, psum_pool, tr_pool, ident,
                x, w_q, w_scales, bias, out, x_slots,
            )
    return nc


def _rep_body(
    nc, tc, const_pool, wq_pool, wd_pool, wt_pool, xt_pool, out_pool,
    psum_pool, tr_pool, ident, x, w_q, w_scales, bias, out, x_slots,
):
    sc_tiles = []
    for o in range(OT):
        sct = const_pool.tile([P, NB], FP32, tag=f"sc_{o}")
        nc.sync.dma_start(sct[:], w_scales.ap()[o * P : (o + 1) * P, :])
        sc_tiles.append(sct)

    wt = wt_pool.tile([P, KT, OUT_SH], BF16, tag="wt")

    def w_og(og):
        rows = slice(og * P, (og + 1) * P)
        wq_i = wq_pool.tile([P, IN], INT32, tag="wq")
        nc.sync.dma_start(wq_i[:], w_q.ap()[rows, :])
        for kg in range(NKG):
            cs = slice(kg * CHUNK, (kg + 1) * CHUNK)
            wd = wd_pool.tile([P, CHUNK], BF16, tag="wd")
            nc.vector.tensor_mul(
                wd[:].rearrange("p (b q) -> p b q", q=QK),
                wq_i[:, cs].rearrange("p (b q) -> p b q", q=QK),
                sc_tiles[og][:, kg * (CHUNK // QK) : (kg + 1) * (CHUNK // QK)]
                .unsqueeze(2)
                .to_broadcast([P, CHUNK // QK, QK]),
            )
            tr = tr_pool.tile([P, KG, P], BF16, tag="tr")
            for j in range(KG):
                nc.tensor.transpose(
                    tr[:, j, :], wd[:, j * P : (j + 1) * P], ident[:]
                )
            nc.scalar.copy(
                wt[:, kg * KG : (kg + 1) * KG, og * P : (og + 1) * P], tr[:]
            )

    def cast_block(b):
        s = b % RING_SLOTS
        srows = slice(b * TOK_BLK, (b + 1) * TOK_BLK)
        return nc.gpsimd.dma_start(x_slots[s].ap()[:, :], x.ap()[srows, :])

    def load_xt(b):
        s = b % RING_SLOTS
        xt = xt_pool.tile([P, KT, TOK_BLK], BF16, tag="xt", name=f"xt_{b}")
        for k in range(KT):
            nc.sync.dma_start(
                xt[:, k, :],
                x_slots[s].ap()[:, k * P : (k + 1) * P],
                transpose=True,
            )
        return xt

    bias_rep = None

    ob_tiles = {}

    def gemm_group(xt, b, m, n, ns):
        tok0 = b * TOK_BLK + m * P
        ps = psum_pool.tile([P, NCOL], FP32, tag="ps")
        for k in range(KT):
            nc.tensor.matmul(
                ps[:],
                xt[:, k, m * P : (m + 1) * P],
                wt[:, k, n * NCOL : (n + 1) * NCOL],
                start=(k == 0),
                stop=(k == KT - 1),
            )
        if (b, m) not in ob_tiles:
            ob_tiles[(b, m)] = out_pool.tile(
                [P, OUT_SH], FP32, tag="ob", name=f"ob_{b}_{m}"
            )
        ob = ob_tiles[(b, m)]
        nc.vector.tensor_add(
            ob[:, n * NCOL : (n + 1) * NCOL],
            ps[:],
            bias_rep[:, n * NCOL : (n + 1) * NCOL],
        )
        if n == ns[-1]:
            # all column chunks of this tok-128 row drained -> one DMA
            del ob_tiles[(b, m)]
            nc.sync.dma_start(out.ap()[tok0 : tok0 + P, :], ob[:])

    # --- stage A: og0-3 + x b0 ---
    cast_block(0)
    cast_block(1)
    for i in range(4):
        w_og(i)
    xt_cache = {0: load_xt(0)}

    bias_rep = const_pool.tile([P, OUT_SH], FP32, tag="bias_rep")
    nc.sync.dma_start(bias_rep[:], bias.ap().unsqueeze(0).to_broadcast([P, OUT_SH]))

    for m in range(MT):
        gemm_group(xt_cache[0], 0, m, 0, [0, 1, 2])

    # --- stage B: og4-7 + x b1 ---
    for i in range(4):
        w_og(4 + i)
    xt_cache[1] = load_xt(1)

    for m in range(MT):
        gemm_group(xt_cache[1], 1, m, 0, [0, 1, 2])

    # --- stage C: og8-11 ---
    last_w = None
    for i in range(4):
        w_og(8 + i)

    # --- main loop ---
    ncast = 2
    for b in range(NBLK):
        if b in xt_cache:
            xt = xt_cache.pop(b)
        else:
            xt = load_xt(b)
        if ncast < NBLK:
            cast_block(ncast)
            ncast += 1
        ns = [1, 2] if b < 2 else [0, 1, 2]
        for n in ns:
            for m in range(MT):
                gemm_group(xt, b, m, n, [0, 1, 2])


_COMPILED_NC = None


def _get_nc():
    global _COMPILED_NC
    if _COMPILED_NC is None:
        import concourse.bacc as bacc

        nc = bacc.Bacc("TRN2", target_bir_lowering=False, debug=False)
        _build(nc)
        nc.compile()
        _COMPILED_NC = nc
    return _COMPILED_NC


def kernel(x, w_q, w_scales, bias):
    from concourse.bass_utils import run_bass_kernel_spmd

    assert x.shape == (TOK, IN) and w_q.shape == (OUT, IN)
    nc = _get_nc()
    x = np.ascontiguousarray(np.asarray(x, dtype=np.float32))
    w_q = np.asarray(w_q, dtype=np.int32)
    w_scales = np.asarray(w_scales, dtype=np.float32)
    bias = np.asarray(bias, dtype=np.float32)
    in_maps = []
    for c in range(N_CORES):
        r = slice(c * OUT_SH, (c + 1) * OUT_SH)
        in_maps.append(
            {
                "x": x,
                "w_q": np.ascontiguousarray(w_q[r]),
                "w_scales": np.ascontiguousarray(w_scales[r]),
                "bias": np.ascontiguousarray(bias[r]),
            }
        )
    res = run_bass_kernel_spmd(nc, in_maps, list(range(N_CORES)))
    return np.concatenate([res.results[c]["out"] for c in range(N_CORES)], axis=1)
